# revision 28
# baseline (speedup 1.0000x reference)
"""LSH similarity-matrix kernel for Trainium2 (8 NeuronCores, data-parallel over batch).

Math: reference computes, per (l, b):
    c1 = (query_embed @ r.T > 0),  c2 = (doc_embed @ r.T > 0)   in {0,1}
    ham = s1 + s2 - 2*c1@c2.T ;  sim = cos(pi/NB * ham), masked where tok==0.
With +-1 codes U = 2c-1 and S = U1 @ U2.T:  ham = (NB - S)/2, so
    sim = sin(pi/(2*NB) * S).
Masks fold into the embeddings: a zeroed embedding row projects to 0,
sign(0) = 0 gives a zero code row, so S = 0 and sin(0) = 0 — exactly the
masked output. Masked doc tokens (half of them: tok in {0,1}) are gathered
away host-side entirely; output columns scatter back as zeros. Batches are
assigned to (core, slot) sorted by active-token count so every core runs an
identically-shaped program with minimal padding per slot.

Structure per batch slot (2 layers):
  one input DMA  ([D, L*pad] doc embeddings, host layout [BPC, D, L, pad]),
  16 single-f32r projection matmuls (8 bit-chunks x 2 layers),
  16 sign ops split across ACT (Sign) / DVE (clamp) — one per chunk, each a
    single instruction over the strided PSUM pieces,
  8  fp8e4m3 DoubleRow code-dot matmuls into one S tile (layers at column
    offsets 0 / 512*np; DoubleRow requires dst partition base 0),
  1  Sin activation over both layers' S columns, fp16 output,
  1  output DMA (host layout [BPC, qpad, L, pad]).
The query side (tiny) runs once, first, so its signs land in the engine
idle window while the first doc DMAs are still in flight.

Precision: both projections run as a SINGLE float32r (TF32-class) matmul
per 128-bit chunk — 1 cycle/row vs fp32's 4 (moving dim >= 256). The f32r
rounding flips hash bits where |proj| falls below the rounding error;
measured on this benchmark data the resulting sim error is <1e-2 relative,
inside the 2e-2 gate. The code dot runs as fp8e4m3 DoubleRow matmuls
(chunk pairs give K=256 per MM at 2 MACs/cell/cycle); +-1/0 codes and
their fp32 PSUM accumulation are exact. sim in [-1,1] makes fp16 output
rounding (2^-11) negligible.

r is pre-scaled by 2^66 host-side so the DVE/gpsimd sign alternative
clamp(x, -1, 1) = max(min(x,1),-1) is exact (any |proj| > 2^-66 maps to
+-1). Embeddings are pre-rounded to tf32 host-side (free) and land via
DMA directly into f32r tiles, so no per-job DVE split/copy work remains.
"""
import os
import sys

sys.path.insert(0, "/opt/trn_rl_repo")

from contextlib import ExitStack

import numpy as np

import concourse.bass as bass
import concourse.mybir as mybir
import concourse.tile as tile
from concourse import bacc
from concourse.bass_utils import run_bass_kernel_spmd

L, BAT, A, BDOC, D, NB = 2, 32, 64, 1024, 128, 1024
CORES = 8
BPC = BAT // CORES          # batch slots per core
CH = NB // 128              # 8 bit-chunks
SCALE = float(2.0 ** 66)
PI = float(np.pi)

F32 = mybir.dt.float32
F32R = mybir.dt.float32r
F16 = mybir.dt.float16
BF16 = mybir.dt.bfloat16
FP8 = mybir.dt.float8e4
Alu = mybir.AluOpType
Act = mybir.ActivationFunctionType

_BUILD_CACHE: dict = {}


def _col_splits(n):
    """Split [0, n) into equal-width pieces of <=512 columns (>=256 keeps
    float32r matmuls at full rate; a matmul may not cross a PSUM bank, so
    piece i is written at PSUM column 512*i). Equal widths mean one strided
    [p, npieces, w] access pattern covers all pieces, so signs run as a
    single instruction per chunk. Returns (c0, c1, p0) per piece."""
    npieces = -(-n // 512)
    w = -(-(n // npieces) // 8) * 8
    while w * npieces < n:
        w += 8
    assert w * npieces >= n and w <= 512
    return [(i * w, min((i + 1) * w, n), 512 * i) for i in range(npieces)]


def _sign(eng, out_ap, in_ap, nc):
    if eng == "dve":
        nc.vector.tensor_scalar(out_ap, in_ap, 1.0, -1.0, Alu.min, Alu.max)
    else:
        nc.scalar.activation(out_ap, in_ap, Act.Sign)


def _chunk_engine(l, k):
    """Per-(layer, chunk) sign engine. Balances ACT (which also runs the
    two per-slot Sins) against DVE: per slot DVE gets 8.5 chunks, ACT 7.5
    (the l=1 chunk-7 pieces split across both)."""
    if k in (0, 2, 4, 6) or (l == 1 and k == 7):
        return "dve"
    return "act"


def _build(pads_c: tuple, qpad: int = A, reps: int = 1, warm_n: int = 4,
           c_first: bool = False, jobp_bufs: int = 3):
    """Per-core SPMD program. pads_c[b]: compute width (mult of 8) of batch
    slot b. reps > 1 re-emits the whole body (timing instrumentation only)."""
    pads_c = tuple(int(p) for p in pads_c)
    pad_cmax = max(pads_c)
    slot_splits = [_col_splits(p) for p in pads_c]
    np_max = max(len(s) for s in slot_splits)

    nc = bacc.Bacc("TRN2", target_bir_lowering=False, debug=False)

    QW = BPC * L * qpad
    QE = nc.dram_tensor("qe", [D, QW], F32R, kind="ExternalInput").ap()
    DE = nc.dram_tensor("de", [BPC, L, D, pad_cmax], F32R, kind="ExternalInput").ap()
    RT = nc.dram_tensor("rt", [D, NB], F32R, kind="ExternalInput").ap()
    OUT = nc.dram_tensor("out", [BPC, qpad, L, pad_cmax], F16,
                         kind="ExternalOutput").ap()

    with tile.TileContext(nc) as tc, ExitStack() as ctx:
        const = ctx.enter_context(tc.tile_pool(name="const", bufs=1))
        jobp = ctx.enter_context(tc.tile_pool(name="jobp", bufs=jobp_bufs))
        outp = ctx.enter_context(tc.tile_pool(name="outp", bufs=2))
        # one rotating PSUM pool shared by projections, query and the code
        # dots: maximum pipeline depth (4 tiles x 2 banks), no dedicated
        # S banks sitting idle
        ps_p = ctx.enter_context(
            tc.tile_pool(name="ps_p", bufs=8 // np_max, space="PSUM"))

        for _rep in range(reps):
            _rp = f"r{_rep}_"
            rh = const.tile([D, NB], F32R, tag="rh", name=f"{_rp}rh")
            qe = const.tile([D, QW], F32R, tag="qe", name=f"{_rp}qe")
            U1 = const.tile([D, CH * QW], FP8, tag="U1", name=f"{_rp}U1")

            _slot_order = sorted(range(BPC), key=lambda s: -pads_c[s])
            st = [dict() for _ in range(BPC)]

            def stage_a_l(si, l):
                b = _slot_order[si]
                pad_c = pads_c[b]
                if l == 0:
                    st[si]["e"] = jobp.tile([D, L * pad_cmax], F32R, tag="e",
                                            name=f"{_rp}e{si}")
                e = st[si]["e"]
                nc.sync.dma_start(out=e[:, l * pad_cmax:l * pad_cmax + pad_c],
                                  in_=DE[b, l, :, 0:pad_c])

            def stage_a(si):
                stage_a_l(si, 0)
                stage_a_l(si, 1)

            # startup DMA order is chosen around the serial DMA-engine and
            # HWDGE pipelines: small const transfers (qe, rh) interleave
            # with the fat per-layer doc transfers so the query projection
            # and slot0/layer0 can start as early as possible
            nc.sync.dma_start(out=rh[:, 0:512], in_=RT[:, 0:512])
            nc.sync.dma_start(out=qe, in_=QE)
            stage_a_l(0, 0)
            nc.sync.dma_start(out=rh[:, 512:NB], in_=RT[:, 512:NB])
            stage_a_l(0, 1)
            if BPC > 1:
                stage_a(1)

            # a 1-column Sin on a zeroed tile runs first so the single
            # activation-table load (every table with sin also has sign)
            # happens in the startup idle window — without it, Sign then Sin
            # trigger two 1.28us loads, the second mid-stream
            tiny = const.tile([D, 8], F32, tag="tiny", name=f"{_rp}tiny")
            nc.vector.memset(tiny[:, 0:8], 0.0)
            nc.scalar.activation(tiny[:, 4:8], tiny[:, 0:4], Act.Sin)

            # PE pre-warm (memset on the otherwise-idle Pool engine):
            # dependency-free matmuls pull the PE through its cold/mid clock
            # ramp while the first DMAs land, so the query projection runs
            # at speed
            if warm_n:
                warm = const.tile([D, 512], BF16, tag="warm", name=f"{_rp}warm")
                nc.gpsimd.memset(warm, 0.0)
                wps = ps_p.tile([D, 512 * np_max], F32, tag="pp",
                                name=f"{_rp}wps")[:, 0:512]
                for i in range(warm_n):
                    nc.tensor.matmul(wps, warm[:, 0:128], warm,
                                     start=True, stop=True)

            def query_proj():
                # runs first: its signs fill the ACT/DVE idle window while
                # the doc DMAs are still landing
                for k in range(CH):
                    rh_k = rh[:, k * 128:(k + 1) * 128]
                    qp = ps_p.tile([D, 512 * np_max], F32, tag="pp",
                                   name=f"{_rp}qp{k}")[:, 0:QW]
                    nc.tensor.matmul(qp, rh_k, qe, start=True, stop=True)
                    u1k = U1[:, k * QW:(k + 1) * QW]
                    _sign("dve" if k % 2 == 0 else "act", u1k, qp, nc)

            def stage_b(si, l):
                # projection + sign for one (slot, layer): 8 chunk matmuls,
                # 8 sign instructions (engine per _chunk_engine)
                b = _slot_order[si]
                pad_c = pads_c[b]
                splits = slot_splits[b]
                npieces = len(splits)
                e = st[si]["e"][:, l * pad_cmax:l * pad_cmax + pad_c]
                if l == 0:
                    st[si]["U2"] = jobp.tile([D, L * CH * pad_cmax], FP8,
                                             tag="U2", name=f"{_rp}U2{si}")
                U2 = st[si]["U2"][:, l * CH * pad_cmax:(l + 1) * CH * pad_cmax]
                for k in range(CH):
                    rh_k = rh[:, k * 128:(k + 1) * 128]
                    pp = ps_p.tile([D, 512 * np_max], F32, tag="pp",
                                   name=f"{_rp}pp{si}_{l}_{k}")
                    for c0, c1, p0 in splits:
                        nc.tensor.matmul(pp[:, p0:p0 + c1 - c0], rh_k,
                                         e[:, c0:c1], start=True, stop=True)
                    eng = _chunk_engine(l, k)
                    w = splits[0][1] - splits[0][0]
                    u2c = U2[:, k * pad_c:(k + 1) * pad_c]
                    if eng == "split":
                        # halves alternate DVE/ACT to fine-balance the slot
                        if npieces == 1:
                            h = (pad_c // 16) * 8
                            _sign("dve", u2c[:, 0:h], pp[:, 0:h], nc)
                            _sign("act", u2c[:, h:pad_c], pp[:, h:pad_c], nc)
                        else:
                            for pi, (c0, c1, p0) in enumerate(splits):
                                _sign("dve" if pi % 2 == 0 else "act",
                                      u2c[:, c0:c1], pp[:, p0:p0 + c1 - c0], nc)
                    elif npieces == 1:
                        _sign(eng, u2c, pp[:, 0:pad_c], nc)
                    elif npieces * w == pad_c:
                        ppv = pp[:].rearrange("p (n c) -> p n c",
                                              c=512)[:, 0:npieces, 0:w]
                        u2v = u2c.rearrange("p (n c) -> p n c", c=w)
                        _sign(eng, u2v, ppv, nc)
                    else:
                        for c0, c1, p0 in splits:
                            _sign(eng, u2c[:, c0:c1],
                                  pp[:, p0:p0 + c1 - c0], nc)

            def stage_c(si, l):
                b = _slot_order[si]
                pad_c = pads_c[b]
                splits = slot_splits[b]
                U2 = st[si]["U2"][:, l * CH * pad_cmax:(l + 1) * CH * pad_cmax]
                # code dot via fp8 DoubleRow into a per-layer S tile drawn
                # from the shared rotating pool (DoubleRow requires dst
                # partition base 0)
                S = ps_p.tile([qpad, 512 * np_max], F32, tag="pp",
                              name=f"{_rp}S{si}_{l}")
                st[si][f"S{l}"] = S
                qcol = (b * L + l) * qpad
                for c0, c1, p0 in splits:
                    w = c1 - c0
                    for jj in range(CH // 2):
                        lw = U1[:, 2 * jj * QW:(2 * jj + 2) * QW] \
                            .rearrange("p (o c) -> p o c", o=2)[:, :, qcol:qcol + qpad]
                        rv = U2[:, 2 * jj * pad_c:(2 * jj + 2) * pad_c] \
                            .rearrange("p (o c) -> p o c", o=2)[:, :, c0:c1]
                        nc.tensor.matmul(
                            S[:, p0:p0 + w], lw, rv,
                            start=(jj == 0), stop=(jj == CH // 2 - 1),
                            perf_mode=mybir.MatmulPerfMode.DoubleRow,
                        )

            def _sin(si, l_, sim, c0, c1, p0):
                b = _slot_order[si]
                S = st[si][f"S{l_}"]
                nc.scalar.activation(
                    sim[:, l_ * pad_cmax + c0:l_ * pad_cmax + c1],
                    S[:, p0:p0 + c1 - c0], Act.Sin, scale=PI / (2.0 * NB))

            def stage_d(si, last=False):
                # per-layer Sin (fp16 out) + one output DMA per slot; the
                # last slot pipelines per-piece sin->DMA so the final DMA
                # latency overlaps the remaining sins
                b = _slot_order[si]
                pad_c = pads_c[b]
                splits = slot_splits[b]
                npieces = len(splits)
                sim = outp.tile([qpad, L * pad_cmax], F16, tag="sim",
                                name=f"{_rp}sim{si}")
                w = splits[0][1] - splits[0][0]
                if last:
                    # per-layer sin -> DMA so the l=0 DMA overlaps the l=1
                    # sin; per-piece DMAs would serialize on HWDGE dispatch
                    for l_ in range(L):
                        for c0, c1, p0 in splits:
                            _sin(si, l_, sim, c0, c1, p0)
                        nc.sync.dma_start(
                            out=OUT[b, :, l_, 0:pad_c],
                            in_=sim[:, l_ * pad_cmax:l_ * pad_cmax + pad_c])
                    return
                for l_ in range(L):
                    S = st[si][f"S{l_}"]
                    mvc = sim[:, l_ * pad_cmax:l_ * pad_cmax + pad_c]
                    if npieces == 1:
                        nc.scalar.activation(mvc, S[:, 0:pad_c], Act.Sin,
                                             scale=PI / (2.0 * NB))
                    elif npieces * w == pad_c:
                        sv = S[:].rearrange("p (n c) -> p n c",
                                            c=512)[:, 0:npieces, 0:w]
                        mv = mvc.rearrange("p (n c) -> p n c", c=w)
                        nc.scalar.activation(mv, sv, Act.Sin,
                                             scale=PI / (2.0 * NB))
                    else:
                        for c0, c1, p0 in splits:
                            _sin(si, l_, sim, c0, c1, p0)
                ov = OUT[b, :, :, 0:pad_c]
                sv2 = sim[:].rearrange("p (o c) -> p o c", o=L)[:, :, 0:pad_c]
                nc.sync.dma_start(out=ov, in_=sv2)

            # ---- emission: query first, then slot-pipelined doc stream.
            # b(si+1) is emitted BEFORE c(si)/d(si): per-engine streams run
            # in emission order, so the PE projects slot si+1 while ACT/DVE
            # drain slot si's signs, and the ACT sin for slot si queues
            # behind slot si+1's signs instead of blocking them. ----
            query_proj()
            stage_b(0, 0)
            stage_b(0, 1)
            for si in range(BPC):
                if si + 2 < BPC:
                    stage_a(si + 2)
                if c_first:
                    stage_c(si, 0)
                    stage_c(si, 1)
                    stage_d(si, last=(si == BPC - 1))
                if si + 1 < BPC:
                    stage_b(si + 1, 0)
                    stage_b(si + 1, 1)
                if not c_first:
                    stage_c(si, 0)
                    stage_c(si, 1)
                    stage_d(si, last=(si == BPC - 1))

    nc.compile()
    return nc


def _tf32(x):
    """Round-to-nearest-even to 11-bit mantissa (bit-matches fp32_to_fp32r)."""
    u = np.ascontiguousarray(x, np.float32).view(np.uint32).astype(np.uint64)
    u = (u + 0x07FF + ((u >> 12) & 1)) & 0xFFFFFFFFFFFFF000
    return (u & 0xFFFFFFFF).astype(np.uint32).view(np.float32)


def _stage_inputs(query_embed, doc_embed, query_tok, doc_tok, r):
    query_embed = np.ascontiguousarray(query_embed, dtype=np.float32)
    doc_embed = np.ascontiguousarray(doc_embed, dtype=np.float32)
    r = np.ascontiguousarray(r, dtype=np.float32)

    qmask = (np.asarray(query_tok) != 0)
    dmask = (np.asarray(doc_tok) != 0)

    # sort batches by active count; slot s takes ranks [s*CORES, (s+1)*CORES)
    # spread across the 8 cores, so per-slot padding is tight and identical
    # on every core (SPMD requires one shape per slot)
    counts = dmask.sum(axis=1).astype(int)
    order = np.argsort(counts, kind="stable")
    assign = np.empty((CORES, BPC), dtype=int)   # assign[c, b] = batch id
    for s in range(BPC):
        for c in range(CORES):
            assign[c, s] = order[s * CORES + c]
    pads_c = tuple(
        min(BDOC, max(64, int(-(-int(counts[assign[:, s]].max()) // 16) * 16)))
        for s in range(BPC)
    )
    pad_cmax = max(pads_c)

    qe_m = query_embed * qmask[None, :, :, None].astype(np.float32)
    qidxs = [np.flatnonzero(qmask[g]) for g in range(BAT)]
    qpad = min(A, max(16, int(-(-max(len(q) for q in qidxs) // 8) * 8)))
    # r is pre-scaled and pre-rounded to tf32 (hw f32r read is then exact)
    rt = _tf32(np.ascontiguousarray(r.T * SCALE))

    idxs = [np.flatnonzero(dmask[g]) for g in range(BAT)]
    in_maps = []
    for c in range(CORES):
        # embeddings staged pre-transposed [D, tokens]; queries compacted
        # to their active rows (masks are per-batch, shared by both layers);
        # embeddings pre-rounded to tf32 so the hw f32r read is exact
        qe_c = np.zeros((D, BPC * L * qpad), dtype=np.float32)
        de_c = np.zeros((BPC, L, D, pad_cmax), dtype=np.float32)
        for b in range(BPC):
            g = assign[c, b]
            qi = qidxs[g]
            for li in range(L):
                col = (b * L + li) * qpad
                qe_c[:, col:col + len(qi)] = qe_m[li, g, qi].T
            idx = idxs[g]
            de_c[b, :, :, :len(idx)] = doc_embed[:, g, idx].transpose(0, 2, 1)
        in_maps.append({"qe": _tf32(qe_c), "de": _tf32(de_c), "rt": rt})

    return in_maps, assign, idxs, pads_c, qidxs, qpad


def kernel(query_embed, doc_embed, query_tok, doc_tok, r):
    in_maps, assign, idxs, pads_c, qidxs, qpad = _stage_inputs(
        query_embed, doc_embed, query_tok, doc_tok, r)

    key = (pads_c, qpad)
    if key not in _BUILD_CACHE:
        _BUILD_CACHE[key] = _build(pads_c, qpad)
    nc = _BUILD_CACHE[key]

    res = run_bass_kernel_spmd(nc, in_maps, core_ids=list(range(CORES)))

    out = np.zeros((BAT, L, A, BDOC), dtype=np.float32)
    for c in range(CORES):
        o_c = res.results[c]["out"]  # [BPC, qpad, L, pad_cmax] fp16 sim
        for b in range(BPC):
            g = assign[c, b]
            idx = idxs[g]
            qi = qidxs[g]
            for li in range(L):
                out[g, li][np.ix_(qi, idx)] = o_c[b, :len(qi), li, :len(idx)]
    return out
